# revision 15
# baseline (speedup 1.0000x reference)
"""Contourlet transform (nn_ContourletCNN) on 8 Trainium2 NeuronCores.

Strategy: pure data parallelism — batch element b -> core b. Per core, each of
the 16 channels runs a 3-level Laplacian pyramid + 3-level directional filter
bank + final wavelet stage:

- Laplacian pyramid (lpdec): separable periodic 5/7-tap filters + dyadic
  up/down-sampling expressed as banded-circulant matmuls on the PE array
  (float32r), with PE transposes between row/col passes.
- DFB qfb stages: the quincunx shear is folded into diagonal DMA access
  patterns reading from horizontally-doubled fp16 DRAM scratch buffers; the
  2-tap diamond filter + downsample becomes a single DVE add/sub of two
  "family" tiles. 'c'-mode stages run in the transposed frame (PE transposes
  between stages).
- Bilinear resize-to-square: a small matmul against an interpolation matrix
  (fp16), using the data-as-lhsT form for transposed subbands so every output
  lands in natural orientation.
- Final wfb2dec: two tiny stacked matmuls.

All 1/sqrt(2) qfb scalings are folded into the resize matrix.
"""

import os
from contextlib import ExitStack

import numpy as np

import concourse.bass as bass
import concourse.mybir as mybir
import concourse.tile as tile
from concourse.vector_clock import VectorClock, ScopedClock
from concourse.bass_utils import run_bass_kernel_spmd

F32 = mybir.dt.float32
F32R = mybir.dt.float32r
F16 = mybir.dt.float16
AOP = mybir.AluOpType

N_CORES = 8
NCH = 16
LEVELS = (512, 256, 128)


# --------------------------------------------------------------------------
# host-side constants
# --------------------------------------------------------------------------

def _pfilters_maxflat():
    s2 = np.sqrt(2.0)
    M1 = M2 = 1.0 / s2
    k1 = 1.0 - s2
    k2 = k3 = M1
    h = np.array([0.25 * k2 * k3, 0.5 * k2, 1.0 + 0.5 * k2 * k3]) * M1
    h = np.concatenate([h, h[:-1][::-1]]) * s2
    g = np.array([-0.125 * k1 * k2 * k3, 0.25 * k1 * k2,
                  -0.5 * k1 - 0.5 * k3 - 0.375 * k1 * k2 * k3,
                  1.0 + 0.5 * k1 * k2]) * M2
    g = np.concatenate([g, g[:-1][::-1]]) * s2
    return h.astype(np.float32), g.astype(np.float32)


def _resize_matrix(n_in):
    # exact replica of jax.image.resize(..., method='bilinear') for 2x upsample
    n_out = 2 * n_in
    U = np.zeros((n_out, n_in), np.float64)
    scale = n_in / n_out
    for y in range(n_out):
        src = (y + 0.5) * scale - 0.5
        for k in range(n_in):
            U[y, k] = max(0.0, 1.0 - abs(src - k))
        U[y] /= U[y].sum()
    return U.astype(np.float32)


def build_consts():
    h, g = _pfilters_maxflat()
    consts = {}
    for S in LEVELS:
        Hd = np.zeros((S // 2, S), np.float32)
        for m in range(S // 2):
            for k in range(5):
                Hd[m, (2 * m + k - 2) % S] += h[k]
        Gu = np.zeros((S, S // 2), np.float32)
        for i in range(S):
            for k in range(7):
                j = (i + k - 3) % S
                if j % 2 == 0:
                    Gu[i, j // 2] += g[k]
        U = _resize_matrix(S // 4) * np.float32(2.0 ** -1.5)
        consts[f"HdT{S}"] = np.ascontiguousarray(Hd.T)                 # [S, S/2] f32
        consts[f"GuT{S}"] = np.ascontiguousarray(Gu.T)                 # [S/2, S] f32
        consts[f"UT{S}"] = np.ascontiguousarray(U.T).astype(np.float16)  # [S/4, S/2] f16
    c = 3
    h1 = (g * ((-1.0) ** (np.arange(len(g)) - c))).astype(np.float32)
    WL = np.zeros((32, 64), np.float32)
    WH = np.zeros((32, 64), np.float32)
    for m in range(32):
        for k in range(5):
            WL[m, (2 * m + k - 2) % 64] += h[k]
        for k in range(7):
            WH[m, (2 * m + 1 + k - 3) % 64] += h1[k]
    WLH = np.concatenate([WL, WH], axis=0)                              # [64, 64]
    consts["WLHT"] = np.ascontiguousarray(WLH.T)
    consts["IDF32"] = np.eye(128, dtype=np.float32)
    consts["IDF16"] = np.eye(128, dtype=np.float16)
    return consts


# --------------------------------------------------------------------------
# TileContext with walrus-friendly tail drain (<=1 sem wait per TPB_CTRL)
# --------------------------------------------------------------------------

class _TC(tile.TileContext):
    def _drain_and_barrier(self, tick_clock, wait_clock):
        gc = tick_clock.global_clock
        for i in range(len(gc)):
            if gc[i] > 0:
                nop_bi = self.nc.sync.nop(nofuse=True)
                vc = VectorClock([0] * len(gc))
                vc.require_at_least(i, gc[i])
                wait_clock.add_sem_waits(nop_bi.ins, ScopedClock({None: vc}))
        self.nc.sync.drain()
        self.nc.all_engine_barrier()
        popped = self.nc._tile_sem_poison_stack.pop()
        assert popped is self._sem_poison
        self.nc.clear_and_free_semaphores(list(self.sems.allocated().values()))
        self.nc.all_engine_barrier()


def _ceil_div(a, b):
    return -(-a // b)


def _wait_limit(inst):
    # this walrus build tolerates exactly one explicit sem wait per instruction
    return 1


def _split_waits(nc):
    """This container's walrus rejects instructions carrying more than a
    per-opcode number of semaphore waits. Move excess waits onto NoOps
    inserted just before the instruction on the same engine (same sequencer
    stream, so ordering and semantics are preserved)."""
    ctr = 0
    for f in nc.m.functions:
        for bb in f.blocks:
            il = bb.instructions
            i = 0
            while i < len(il):
                inst = il[i]
                si = inst.sync_info
                limit = _wait_limit(inst)
                if si is not None and si.on_wait and len(si.on_wait) > limit:
                    waits = list(si.on_wait)
                    excess, keep = waits[:-limit], waits[-limit:]
                    pos = i
                    for j in range(0, len(excess), limit):
                        chunk = excess[j:j + limit]
                        nop = mybir.InstNoOp(name=f"wsplit_{ctr}")
                        ctr += 1
                        nop.engine = inst.engine
                        nop.sync_info = mybir.SyncInfo(on_wait=chunk, on_update=[])
                        il.insert(pos, nop)
                        pos += 1
                        i += 1
                    inst.sync_info = mybir.SyncInfo(on_wait=keep,
                                                    on_update=list(si.on_update))
                i += 1


# --------------------------------------------------------------------------
# program builder
# --------------------------------------------------------------------------

class Builder:
    def __init__(self, nc, tc, ctx, nch=NCH):
        self.nc = nc
        self.tc = tc
        self.ctx = ctx
        self.nch = nch
        self.evac_ctr = 0
        self.dma_ctr = 0
        self.pools = {}
        self.const_tiles = {}

    # ---- engine rotation helpers ----
    def dma(self, out, in_):
        eng = (self.nc.sync, self.nc.scalar)[self.dma_ctr % 2]
        self.dma_ctr += 1
        eng.dma_start(out=out, in_=in_)

    def evac(self, out, in_):
        """PSUM -> SBUF copy (casts via AP dtypes)."""
        if self.evac_ctr % 2 == 0:
            self.nc.scalar.copy(out, in_)
        else:
            self.nc.vector.tensor_copy(out, in_)
        self.evac_ctr += 1

    # ---- tile helpers ----
    def work_tile(self, shape, dtype, tag):
        return self.pools["work"].tile(shape, dtype, tag=tag, name=tag)

    def psum_tile(self, shape, dtype):
        return self.pools["psum"].tile(shape, dtype, tag="ps", name="ps")

    # ---- matmul: OUT[P,N] = M[P,Q] @ X[Q,N]; MT_tile = M^T block-major ----
    def emit_mm(self, MT_tile, P, Q, X_tile, N, writefn, fp32=True):
        nc = self.nc
        nob = _ceil_div(P, 128)
        nkt = _ceil_div(Q, 128)
        for ob in range(nob):
            pm = min(128, P - ob * 128)
            ps = self.psum_tile([pm, N], F32)
            for kt in range(nkt):
                pk = min(128, Q - kt * 128)
                lhsT = MT_tile[0:pk, kt * P + ob * 128: kt * P + ob * 128 + pm]
                rhs = X_tile[0:pk, kt * N:(kt + 1) * N]
                nc.tensor.matmul(ps[:, :], lhsT, rhs,
                                 start=(kt == 0), stop=(kt == nkt - 1))
            writefn(ob, pm, ps)

    # ---- transpose: OUT[Cd,R] = X[R,Cd]^T (block-major both sides) ----
    # fp32 path runs the PE in plain-float32 transpose mode (tolerates pitched
    # inputs); fp16 runs in fp16 mode, which requires densely packed weights,
    # so each block is staged into a packed [128,128] scratch via GPSIMD.
    def emit_transpose(self, X_tile, R, Cd, dtype, out_tile):
        nc = self.nc
        fp32 = dtype in (F32, F32R)
        ncb = _ceil_div(Cd, 128)
        nrb = _ceil_div(R, 128)
        for ocb in range(ncb):
            pc = min(128, Cd - ocb * 128)
            if fp32:
                ident = self.const_tiles["IDF32"]
                ps = self.psum_tile([pc, R], F32)
                for rb in range(nrb):
                    pr = min(128, R - rb * 128)
                    in_blk = X_tile[0:pr,
                                    rb * Cd + ocb * 128: rb * Cd + ocb * 128 + pc]
                    nc.tensor.transpose(ps[0:pc, rb * 128: rb * 128 + pr],
                                        in_blk.bitcast(F32),
                                        ident[0:pr, 0:pr].bitcast(F32))
                self.evac(out_tile[0:pc, ocb * R:(ocb + 1) * R], ps[0:pc, 0:R])
            else:
                ident = self.const_tiles["IDF16"]
                ps = self.psum_tile([128, nrb * 128], F16)
                for rb in range(nrb):
                    pr = min(128, R - rb * 128)
                    in_blk = X_tile[0:pr,
                                    rb * Cd + ocb * 128: rb * Cd + ocb * 128 + pc]
                    stage = self.pools["z"].tile([128, 128], F16,
                                                 tag="tstage16", name="tstage16")
                    nc.gpsimd.tensor_copy(stage[0:pr, 0:pc], in_blk)
                    nc.tensor.transpose(ps[:, rb * 128: rb * 128 + 128],
                                        stage[:, :], ident[:, :])
                self.evac(out_tile[0:pc, ocb * R:(ocb + 1) * R], ps[0:pc, 0:R])

    # ---- DMA store: X[R,Cw] (block-major tile) -> dram natural, 1-2 DMAs ----
    # SBUF AP rule: only dim0 may advance partitions; <=3 dims per DMA.
    # Full 128-row blocks move in ONE DMA with the block dim as a mid free
    # dim on the SBUF side / a strided dim on the DRAM side.
    def store_natural(self, X_tile, R, Cw, dram, elem_off):
        nb = R // 128
        pitch = _ceil_div(R, 128) * Cw
        if nb:
            src = bass.AP(X_tile.tensor, X_tile.offset,
                          [[pitch, 128], [Cw, nb], [1, Cw]])
            dst = bass.AP(dram, elem_off, [[Cw, 128], [128 * Cw, nb], [1, Cw]])
            self.dma(out=dst, in_=src)
        pr = R - nb * 128
        if pr:
            src = X_tile[0:pr, nb * Cw:(nb + 1) * Cw]
            dst = bass.AP(dram, elem_off + nb * 128 * Cw, [[Cw, pr], [1, Cw]])
            self.dma(out=dst, in_=src)

    # ---- DMA store doubled + wrap pad row: X[R,Cw] -> buf[ch] [R+1, 2Cw] ----
    def store_doubled(self, X_tile, R, Cw, dram, ch):
        rp = 2 * Cw
        base = ch * (R + 1) * rp
        nb = R // 128
        pitch = _ceil_div(R, 128) * Cw
        if nb == 1 and R == 128:
            # single block: both copies in one DMA via a 0-step source dim
            src = bass.AP(X_tile.tensor, X_tile.offset,
                          [[pitch, R], [0, 2], [1, Cw]])
            dst = bass.AP(dram, base, [[rp, R], [Cw, 2], [1, Cw]])
            self.dma(out=dst, in_=src)
        else:
            for cp in range(2):
                if nb:
                    src = bass.AP(X_tile.tensor, X_tile.offset,
                                  [[pitch, 128], [Cw, nb], [1, Cw]])
                    dst = bass.AP(dram, base + cp * Cw,
                                  [[rp, 128], [128 * rp, nb], [1, Cw]])
                    self.dma(out=dst, in_=src)
                pr = R - nb * 128
                if pr:
                    src = X_tile[0:pr, nb * Cw:(nb + 1) * Cw]
                    dst = bass.AP(dram, base + cp * Cw + nb * 128 * rp,
                                  [[rp, pr], [1, Cw]])
                    self.dma(out=dst, in_=src)
        # pad row R = copy of row 0, both halves in one DMA
        dst = bass.AP(dram, base + R * rp, [[Cw, 2], [1, Cw]])
        src = bass.AP(X_tile.tensor, X_tile.offset, [[pitch, 1], [0, 2], [1, Cw]])
        self.dma(out=dst, in_=src)

    # ---- load a [R,Cw] DRAM region into a block-major tile, 1-2 DMAs ----
    def load_blockmajor(self, tile_ap, R, Cw, dram, elem_off):
        nb = R // 128
        pitch = _ceil_div(R, 128) * Cw
        if nb:
            dst = bass.AP(tile_ap.tensor, tile_ap.offset,
                          [[pitch, 128], [Cw, nb], [1, Cw]])
            src = bass.AP(dram, elem_off, [[Cw, 128], [128 * Cw, nb], [1, Cw]])
            self.dma(out=dst, in_=src)
        pr = R - nb * 128
        if pr:
            dst = tile_ap[0:pr, nb * Cw:(nb + 1) * Cw]
            src = bass.AP(dram, elem_off + nb * 128 * Cw, [[Cw, pr], [1, Cw]])
            self.dma(out=dst, in_=src)

    # ---- qfb stage: diagonal fam loads + DVE pair add/sub ----
    def emit_stage(self, dram_buf, ch, H, W, s):
        """Input: doubled buffer [nch, H+1, 2W] fp16 holding Y [H, W].
        Returns (lo, hi) tiles, each [H/2, W] block-major fp16:
        lo[m] = z[2m] + z[2m+1], hi[m] = z[2m+1] - z[2m+2],
        z[i,j] = Y[i, (j + s*i) % W]."""
        nc = self.nc
        M = H // 2
        nb2 = _ceil_div(M, 128)
        FAMSZ = nb2 * W
        pdim = min(128, M)
        rp = 2 * W
        base = ch * (H + 1) * rp
        zt = self.pools["z"].tile([pdim, 3 * FAMSZ], F16, tag=f"z{H}_{W}", name=f"z{H}_{W}")
        pitch = 3 * FAMSZ
        # greedy "extended" shear shifts in [0, W] (the doubled buffer makes
        # shift == W valid), keeping address runs linear through wrap points
        def _canon(i):
            v = (s * i) % W
            return W if (s < 0 and v == 0) else v
        shift_seq = [_canon(0)]
        for i in range(1, H + 1):
            cand = shift_seq[-1] + s
            shift_seq.append(cand if 0 <= cand <= W else _canon(i))
        for b2 in range(nb2):
            m0 = b2 * 128
            m1 = min(M, m0 + 128)
            A = [[(2 * m + rgn) * rp + shift_seq[2 * m + rgn]
                  for m in range(m0, m1)] for rgn in range(3)]
            n = m1 - m0
            # segment rows: a row m is "mergeable" into a run when all three
            # region addresses advance with constant strides
            r0 = 0
            while r0 < n:
                d01 = A[1][r0] - A[0][r0]
                d12 = A[2][r0] - A[1][r0]
                merged = d01 == d12
                r1 = r0 + 1
                mstep = (A[0][r0 + 1] - A[0][r0]) if r0 + 1 < n else rp
                while r1 < n:
                    ok = (A[1][r1] - A[0][r1] == d01 and
                          A[2][r1] - A[1][r1] == d12 and
                          A[0][r1] - A[0][r1 - 1] == mstep and
                          A[1][r1] - A[1][r1 - 1] == mstep and
                          A[2][r1] - A[2][r1 - 1] == mstep)
                    if not ok:
                        break
                    r1 += 1
                cnt = r1 - r0
                if merged:
                    src = bass.AP(dram_buf, base + A[0][r0],
                                  [[mstep, cnt], [d01, 3], [1, W]])
                    dst = bass.AP(zt.tensor, zt.offset + r0 * pitch + b2 * W,
                                  [[pitch, cnt], [FAMSZ, 3], [1, W]])
                    self.dma(out=dst, in_=src)
                else:
                    for rgn in range(3):
                        src = bass.AP(dram_buf, base + A[rgn][r0],
                                      [[mstep, cnt], [1, W]])
                        dst = zt[r0:r0 + cnt,
                                 rgn * FAMSZ + b2 * W: rgn * FAMSZ + b2 * W + W]
                        self.dma(out=dst, in_=src)
                r0 = r1
        lo = self.pools["z"].tile([pdim, FAMSZ], F16, tag=f"lo{H}_{W}", name=f"lo{H}_{W}")
        hi = self.pools["z"].tile([pdim, FAMSZ], F16, tag=f"hi{H}_{W}", name=f"hi{H}_{W}")
        Fz = FAMSZ
        nc.vector.tensor_add(lo[0:pdim, :], zt[0:pdim, 0:Fz], zt[0:pdim, Fz:2 * Fz])
        nc.vector.tensor_sub(hi[0:pdim, :], zt[0:pdim, Fz:2 * Fz], zt[0:pdim, 2 * Fz:3 * Fz])
        return lo, hi

    # ---- lpdec for one channel at size S ----
    def emit_lpdec(self, ch, S, cur_tile, last):
        nc = self.nc
        nb = _ceil_div(S, 128)
        Sh = S // 2
        nbh = _ceil_div(Sh, 128)
        HdT = self.const_tiles[f"HdT{S}"]
        GuT = self.const_tiles[f"GuT{S}"]

        R1 = self.work_tile([min(128, Sh), nbh * S], F32R, f"R1_{S}")
        self.emit_mm(HdT, Sh, S, cur_tile, S,
                     lambda ob, pm, ps: self.evac(R1[0:pm, ob * S:(ob + 1) * S], ps[0:pm, :]))
        T1 = self.work_tile([128, nb * Sh], F32R, f"T1_{S}")
        self.emit_transpose(R1, Sh, S, F32R, T1)
        xloT = self.work_tile([min(128, Sh), nbh * Sh], F32R, f"xloT_{S}")
        self.emit_mm(HdT, Sh, S, T1, Sh,
                     lambda ob, pm, ps: self.evac(xloT[0:pm, ob * Sh:(ob + 1) * Sh], ps[0:pm, :]))
        V = self.work_tile([128, nb * Sh], F32R, f"V_{S}")
        self.emit_mm(GuT, S, Sh, xloT, Sh,
                     lambda ob, pm, ps: self.evac(V[0:pm, ob * Sh:(ob + 1) * Sh], ps[0:pm, :]))
        W1 = self.work_tile([min(128, Sh), nbh * S], F32R, f"W1_{S}")
        self.emit_transpose(V, S, Sh, F32R, W1)
        xhi = self.work_tile([128, nb * S], F16, f"xhi_{S}")

        def xhi_write(ob, pm, ps):
            nc.vector.scalar_tensor_tensor(
                out=xhi[0:pm, ob * S:(ob + 1) * S],
                in0=ps[0:pm, :], scalar=-1.0,
                in1=cur_tile[0:pm, ob * S:(ob + 1) * S].bitcast(F32),
                op0=AOP.mult, op1=AOP.add)

        self.emit_mm(GuT, S, Sh, W1, S, xhi_write)

        if last:
            return xhi, xloT
        nxt = self.work_tile([min(128, Sh), nbh * Sh], F32R, f"cur_{Sh}")
        self.emit_transpose(xloT, Sh, Sh, F32R, nxt)
        return xhi, nxt

    # ---- dfb + resize + store for one channel at size S ----
    def emit_dfb(self, ch, S, xhi, bufs, fout, fout_stride):
        Sh = S // 2
        Sq = S // 4
        self.store_doubled(xhi, S, S, bufs["hi"], ch)
        # stage A ('r', +1) on xhi
        x0, x1 = self.emit_stage(bufs["hi"], ch, S, S, +1)
        for t, key in ((x0, "x0T"), (x1, "x1T")):
            tT = self.work_tile([128, _ceil_div(S, 128) * Sh], F16, f"sT_{S}")
            self.emit_transpose(t, Sh, S, F16, tT)
            self.store_doubled(tT, S, Sh, bufs[key], ch)
        # stage B ('r' frame of x0^T, +1) -> a0^T, a1^T; transpose to natural
        loB, hiB = self.emit_stage(bufs["x0T"], ch, S, Sh, +1)
        for t, key in ((loB, "a0"), (hiB, "a1")):
            tT = self.work_tile([min(128, Sh), _ceil_div(Sh, 128) * Sh], F16, f"aT_{S}")
            self.emit_transpose(t, Sh, Sh, F16, tT)
            self.store_doubled(tT, Sh, Sh, bufs[key], ch)
        # stage C ('r' frame of x1^T, -1) -> a2^T, a3^T stored directly
        loC, hiC = self.emit_stage(bufs["x1T"], ch, S, Sh, -1)
        self.store_doubled(loC, Sh, Sh, bufs["a2T"], ch)
        self.store_doubled(hiC, Sh, Sh, bufs["a3T"], ch)
        # stages D..G
        y0, y1 = self.emit_stage(bufs["a0"], ch, Sh, Sh, +1)
        y2, y3 = self.emit_stage(bufs["a1"], ch, Sh, Sh, -1)
        y4T, y5T = self.emit_stage(bufs["a2T"], ch, Sh, Sh, +1)
        y6T, y7T = self.emit_stage(bufs["a3T"], ch, Sh, Sh, -1)
        # resize + store; final order [y0,y1,y2,y3,y7,y6,y5,y4]
        plan = [(y0, 0, False), (y1, 1, False), (y2, 2, False), (y3, 3, False),
                (y4T, 7, True), (y5T, 6, True), (y6T, 5, True), (y7T, 4, True)]
        UT = self.const_tiles[f"UT{S}"]
        for yt, k, transposed in plan:
            outsb = self.work_tile([min(128, Sh), _ceil_div(Sh, 128) * Sh], F32, f"rs_{S}")
            for ob in range(_ceil_div(Sh, 128)):
                pm = min(128, Sh - ob * 128)
                ps = self.psum_tile([pm, Sh], F32)
                if transposed:
                    lhsT = yt[0:Sq, ob * 128: ob * 128 + pm]
                    rhs = UT[0:Sq, 0:Sh]
                else:
                    lhsT = UT[0:Sq, ob * 128: ob * 128 + pm]
                    rhs = yt[0:Sq, 0:Sh]
                self.nc.tensor.matmul(ps[:, :], lhsT, rhs, start=True, stop=True)
                self.evac(outsb[0:pm, ob * Sh:(ob + 1) * Sh], ps[0:pm, :])
            self.store_natural(outsb, Sh, Sh, fout, (k * self.nch + ch) * fout_stride)

    # ---- final wavelet stage on xloT3 [64, 64] f32 ----
    def emit_wfb(self, ch, xloT3, f0):
        nc = self.nc
        WLHT = self.const_tiles["WLHT"]
        ps1 = self.psum_tile([64, 64], F32)
        nc.tensor.matmul(ps1[:, :], WLHT[0:64, 0:64],
                         xloT3[0:64, 0:64], start=True, stop=True)
        st1 = self.work_tile([64, 64], F32R, "wfb_st1")
        self.evac(st1[0:64, 0:64], ps1[0:64, :])
        st1t = self.work_tile([64, 64], F32R, "wfb_st1t")
        self.emit_transpose(st1, 64, 64, F32R, st1t)
        ps2 = self.psum_tile([64, 64], F32)
        nc.tensor.matmul(ps2[:, :], WLHT[0:64, 0:64],
                         st1t[0:64, 0:64], start=True, stop=True)
        out2 = self.work_tile([64, 64], F32, "wfb_out")
        self.evac(out2[0:64, 0:64], ps2[0:64, :])
        # quadrants: (row0, col0) -> k: (0,0)->0 xLL, (32,0)->1 xLH, (0,32)->2 xHL, (32,32)->3 xHH
        for (r0, c0, k) in ((0, 0, 0), (32, 0, 1), (0, 32, 2), (32, 32, 3)):
            dst = bass.AP(f0, (k * self.nch + ch) * 32 * 32, [[32, 32], [1, 32]])
            self.dma(out=dst, in_=out2[r0:r0 + 32, c0:c0 + 32])


def build_program(consts, nch=NCH, levels=LEVELS):
    nc = bass.Bass("TRN2", target_bir_lowering=False, debug=False,
                   num_devices=N_CORES)
    x_in = nc.dram_tensor("x", [nch, 512, 512], F32, kind="ExternalInput").ap()
    const_dram = {}
    for name, arr in consts.items():
        dt = F16 if arr.dtype == np.float16 else F32
        const_dram[name] = nc.dram_tensor(name, list(arr.shape), dt,
                                          kind="ExternalInput").ap()
    outs = {}
    out_shapes = {"f0": [4 * nch, 32, 32]}
    for S in levels:
        outs_name = {512: "f1", 256: "f2", 128: "f3"}[S]
        out_shapes[outs_name] = [8 * nch, S // 2, S // 2]
    for name, shp in out_shapes.items():
        outs[name] = nc.dram_tensor(name, shp, F32, kind="ExternalOutput").ap()

    # per-level DRAM scratch (doubled fp16 buffers, +1 wrap pad row)
    lvbufs = {}
    for S in levels:
        Sh = S // 2
        b = {}
        b["hi"] = nc.dram_tensor(f"bufHI{S}", [nch, S + 1, 2 * S], F16).ap().tensor
        for key, rows, w in (("x0T", S, Sh), ("x1T", S, Sh),
                             ("a0", Sh, Sh), ("a1", Sh, Sh),
                             ("a2T", Sh, Sh), ("a3T", Sh, Sh)):
            b[key] = nc.dram_tensor(f"buf{key}{S}", [nch, rows + 1, 2 * w], F16).ap().tensor
        b["hi"] = b["hi"]
        lvbufs[S] = b

    with _TC(nc) as tc, ExitStack() as ctx:
        bld = Builder(nc, tc, ctx, nch=nch)
        bld.pools["const"] = ctx.enter_context(tc.tile_pool(name="const", bufs=1))
        bld.pools["work"] = ctx.enter_context(tc.tile_pool(name="work", bufs=2))
        bld.pools["z"] = ctx.enter_context(tc.tile_pool(name="z", bufs=2))
        bld.pools["psum"] = ctx.enter_context(
            tc.tile_pool(name="psum", bufs=8, space="PSUM"))

        # load constants (fp32 consts get a rounding copy to float32r for PE)
        for name, arr in consts.items():
            R, Cw = arr.shape
            is16 = arr.dtype == np.float16
            dt = F16 if is16 else F32
            nb = _ceil_div(R, 128)
            t = bld.pools["const"].tile([min(128, R), nb * Cw], dt, tag=f"c_{name}", name=f"c_{name}")
            bld.load_blockmajor(t, R, Cw, const_dram[name].tensor, 0)
            if is16:
                bld.const_tiles[name] = t
            else:
                tr = bld.pools["const"].tile([min(128, R), nb * Cw], F32R,
                                             tag=f"cr_{name}", name=f"cr_{name}")
                nc.vector.tensor_copy(tr[0:min(128, R), :], t[0:min(128, R), :])
                bld.const_tiles[name] = tr

        fmap = {512: "f1", 256: "f2", 128: "f3"}
        for ch in range(nch):
            cur_raw = bld.work_tile([128, 4 * 512], F32, "cur_raw")
            bld.load_blockmajor(cur_raw, 512, 512, x_in.tensor, ch * 512 * 512)
            cur = bld.work_tile([128, 4 * 512], F32R, "cur_512")
            nc.vector.tensor_copy(cur[:, :], cur_raw[:, :])
            for li, S in enumerate(levels):
                last = li == len(levels) - 1
                xhi, nxt = bld.emit_lpdec(ch, S, cur, last)
                bld.emit_dfb(ch, S, xhi, lvbufs[S], outs[fmap[S]].tensor,
                             (S // 2) * (S // 2))
                cur = nxt
            bld.emit_wfb(ch, cur, outs["f0"].tensor)
    _split_waits(nc)
    return nc


# --------------------------------------------------------------------------
# public entry point
# --------------------------------------------------------------------------

_CACHE = {}


def kernel(x):
    x = np.ascontiguousarray(np.asarray(x), dtype=np.float32)
    assert x.shape == (8, 16, 512, 512), x.shape
    if "nc" not in _CACHE:
        consts = build_consts()
        _CACHE["consts"] = consts
        _CACHE["nc"] = build_program(consts)
    consts = _CACHE["consts"]
    nc = _CACHE["nc"]
    in_maps = []
    for i in range(N_CORES):
        m = {"x": np.ascontiguousarray(x[i])}
        m.update(consts)
        in_maps.append(m)
    res = run_bass_kernel_spmd(nc, in_maps, list(range(N_CORES)))
    f0 = np.stack([res.results[i]["f0"] for i in range(N_CORES)])
    f1 = np.stack([res.results[i]["f1"] for i in range(N_CORES)])
    f2 = np.stack([res.results[i]["f2"] for i in range(N_CORES)])
    f3 = np.stack([res.results[i]["f3"] for i in range(N_CORES)])
    return f0, f3, f2, f1
